# revision 1
# baseline (speedup 1.0000x reference)
"""GNN message passing (HJRLConv) on 8 Trainium2 NeuronCores.

out = relu(segment_sum(edge_vals * (X @ W)[edge_src], edge_dst))
    = relu((segment_sum(edge_vals * X[edge_src], edge_dst)) @ W)

Sharding: destination nodes row-partitioned across 8 cores (12500 rows each);
edges bucketed by destination partition on the host; X replicated in bf16
(each core gathers source rows from its own full copy in local HBM, so no
halo-exchange collective is needed).

Per core:
  - edges grouped by 128-row destination block and 32768-row source range
    (dma_gather indices are int16), sorted by source within each bucket for
    HBM locality, padded to chunks of 128 edges
  - SWDGE dma_gather fetches X_bf16[src]; gathers are split into <=12-chunk
    pieces so each fits the (enlarged, 48KB) descriptor ring without
    stalling the Pool sequencer, round-robined over 4 SWDGE queues, and
    buffered 16 deep so DMA runs far ahead of compute
  - an indicator matrix sv[e, d] = val[e] * (dst_rel[e] == d) is built on
    DVE with one scalar_tensor_tensor per chunk. (Crucially NOT
    tensor_scalar: 1-input DVE ops enter 2-port perf mode, which locks
    GPSIMD out of the shared SBUF port pair and starves SWDGE descriptor
    generation - the tensor_tensor family never contends.)
  - PE matmul xg.T @ sv accumulates aggT[feat, dst] per dst block in PSUM
  - final bf16 matmul aggT.T @ W, ReLU on ACT, DMA to DRAM

The chunk schedule is derived from the actual edge data and baked into the
compiled program; it is shared by all 8 cores (max over cores per
(block, range)), with val=0 padding edges keeping the program SPMD-uniform.
"""

import functools

import numpy as np
import ml_dtypes

import concourse.bacc as bacc
import concourse.bass as bass
import concourse.tile as tile
from concourse import library_config, mybir
from concourse.bass_utils import run_bass_kernel_spmd

N_NODES = 100000
N_EDGES = 1600000
D = 128
N_CORES = 8
ROWS_PER_CORE = N_NODES // N_CORES  # 12500
N_BLOCKS = (ROWS_PER_CORE + 127) // 128  # 98
PAD_ROWS = N_BLOCKS * 128  # 12544
RANGE = 32768  # dma_gather int16 index limit
N_RANGES = (N_NODES + RANGE - 1) // RANGE  # 4
SUPER = 6  # blocks per super-block (6 agg PSUM banks + 2 out banks = 8)
GMAX = 12  # max chunks per gather instruction (1536 descriptors)
SCRATCH = 49152  # SWDGE descriptor ring bytes/partition (3072 descs/queue)
N_QUEUES = 4
XG_BUFS = 16
SV_BUFS = 32


def _chunk_layout(cpbr):
    """Linear chunk order: super-blocks of SUPER blocks; within one,
    range-major then block-major. Returns (tot, chunk_off[b, r])."""
    chunk_off = np.zeros((N_BLOCKS, N_RANGES), dtype=np.int64)
    pos = 0
    for s0 in range(0, N_BLOCKS, SUPER):
        blocks = range(s0, min(s0 + SUPER, N_BLOCKS))
        for r in range(N_RANGES):
            for b in blocks:
                chunk_off[b, r] = pos
                pos += cpbr[b, r]
    return int(pos), chunk_off


def _schedule(edge_src, edge_vals, edge_dst):
    core = edge_dst // ROWS_PER_CORE
    counts = np.zeros((N_CORES, N_BLOCKS * N_RANGES), dtype=np.int64)
    per_core = []
    for c in range(N_CORES):
        sel = np.nonzero(core == c)[0]
        dst_l = edge_dst[sel] - c * ROWS_PER_CORE
        key = (dst_l >> 7) * N_RANGES + (edge_src[sel] >> 15)
        # sort by bucket, then by src within the bucket (HBM locality)
        order = np.lexsort((edge_src[sel], key))
        sel = sel[order]
        key = key[order]
        counts[c] = np.bincount(key, minlength=N_BLOCKS * N_RANGES)
        per_core.append((sel, key, ((edge_dst[sel] - c * ROWS_PER_CORE) & 127)))

    cpbr = -(-counts.max(axis=0).reshape(N_BLOCKS, N_RANGES) // 128)  # ceil
    empty = cpbr.sum(axis=1) == 0
    cpbr[empty, 0] = 1  # every block needs >=1 chunk to produce output
    tot, chunk_off = _chunk_layout(cpbr)

    idx16 = np.zeros((N_CORES, 128, tot * 8), dtype=np.int16)
    dst_T = np.zeros((N_CORES, 128, tot), dtype=np.float32)
    val_T = np.zeros((N_CORES, 128, tot), dtype=np.float32)
    slot_start = chunk_off.reshape(-1) * 128  # by key
    for c in range(N_CORES):
        sel, key, dst_rel = per_core[c]
        cnt = counts[c]
        key_start_sorted = np.concatenate([[0], np.cumsum(cnt)[:-1]])
        rank = np.arange(len(sel)) - key_start_sorted[key]
        pos = slot_start[key] + rank
        idx_flat = np.zeros(tot * 128, dtype=np.int16)
        dst_flat = np.zeros(tot * 128, dtype=np.float32)
        val_flat = np.zeros(tot * 128, dtype=np.float32)
        idx_flat[pos] = (edge_src[sel] & (RANGE - 1)).astype(np.int16)
        dst_flat[pos] = dst_rel
        val_flat[pos] = edge_vals[sel]
        # dma_gather wrapped index layout: index i -> [i % 16, i // 16],
        # replicated across the 8 groups of 16 partitions
        wrapped = idx_flat.reshape(tot * 8, 16).T  # [16, tot*8]
        idx16[c] = np.tile(wrapped, (8, 1))
        dst_T[c] = dst_flat.reshape(tot, 128).T
        val_T[c] = val_flat.reshape(tot, 128).T
    return cpbr, tot, idx16, dst_T, val_T


@functools.lru_cache(maxsize=4)
def _build_program(cpbr_key, repeat=1):
    cpbr = np.asarray(cpbr_key, dtype=np.int64).reshape(N_BLOCKS, N_RANGES)
    tot, chunk_off = _chunk_layout(cpbr)
    nch_block = cpbr.sum(axis=1)

    nc = bacc.Bacc("TRN2", target_bir_lowering=False, debug=False,
                   num_devices=N_CORES, num_swdge_queues=N_QUEUES,
                   dynamic_dma_scratch_size=SCRATCH)
    bf16 = mybir.dt.bfloat16
    f32 = mybir.dt.float32

    x_t = nc.dram_tensor("xbf", [N_NODES, D], bf16, kind="ExternalInput")
    w_t = nc.dram_tensor("w", [D, D], f32, kind="ExternalInput")
    iota_t = nc.dram_tensor("iota", [128, 128], bf16, kind="ExternalInput")
    idx_t = nc.dram_tensor("idx", [128, tot * 8], mybir.dt.int16,
                           kind="ExternalInput")
    dst_t = nc.dram_tensor("dstrel", [128, tot], f32, kind="ExternalInput")
    val_t = nc.dram_tensor("val", [128, tot], f32, kind="ExternalInput")
    out_t = nc.dram_tensor("out", [PAD_ROWS, D], f32, kind="ExternalOutput")

    with tile.TileContext(nc) as tc:
        with (
            tc.tile_pool(name="const", bufs=1) as cpool,
            tc.tile_pool(name="meta", bufs=1) as mpool,
            tc.tile_pool(name="xg", bufs=XG_BUFS) as xgpool,
            tc.tile_pool(name="sv", bufs=SV_BUFS) as svpool,
            tc.tile_pool(name="agg", bufs=6) as aggpool,
            tc.tile_pool(name="osb", bufs=6) as opool,
            tc.tile_pool(name="psA", bufs=SUPER, space="PSUM") as psa,
            tc.tile_pool(name="psB", bufs=2, space="PSUM") as psb,
        ):
            nc.gpsimd.load_library(library_config.mlp)
            w_sb = cpool.tile([128, 128], f32, tag="w")
            nc.sync.dma_start(out=w_sb[:], in_=w_t.ap())
            w_bf = cpool.tile([128, 128], bf16, tag="wbf")
            nc.vector.tensor_copy(out=w_bf[:], in_=w_sb[:])
            iota_sb = cpool.tile([128, 128], bf16, tag="iota")
            nc.sync.dma_start(out=iota_sb[:], in_=iota_t.ap())
            idx_sb = mpool.tile([128, tot * 8], mybir.dt.int16, tag="idx")
            nc.sync.dma_start(out=idx_sb[:], in_=idx_t.ap())
            dst_sb = mpool.tile([128, tot], f32, tag="dst")
            nc.sync.dma_start(out=dst_sb[:], in_=dst_t.ap())
            val_sb = mpool.tile([128, tot], f32, tag="val")
            nc.sync.dma_start(out=val_sb[:], in_=val_t.ap())

            qctr = [0]
            for _rep in range(repeat):
              for s0 in range(0, N_BLOCKS, SUPER):
                blocks = list(range(s0, min(s0 + SUPER, N_BLOCKS)))
                # gathers per source range covering this super-block's
                # chunks (contiguous in the layout), split to <=GMAX chunks
                gathers = {}  # r -> (sub tiles, first_chunk)
                for r in range(N_RANGES):
                    nch = int(sum(cpbr[b, r] for b in blocks))
                    if nch == 0:
                        continue
                    first = int(chunk_off[blocks[0], r])
                    base = r * RANGE
                    rows = min(RANGE, N_NODES - base)
                    subs = []
                    for s_off in range(0, nch, GMAX):
                        snch = min(GMAX, nch - s_off)
                        sfirst = first + s_off
                        xg = xgpool.tile([128, GMAX, 128], bf16, tag="xg")
                        nc.gpsimd.dma_gather(
                            out_ap=xg[:, :snch, :],
                            in_ap=x_t.ap()[base : base + rows, :],
                            idxs_ap=idx_sb[:, sfirst * 8
                                           : (sfirst + snch) * 8],
                            num_idxs=snch * 128,
                            num_idxs_reg=snch * 128,
                            elem_size=D,
                            single_packet=False,
                            queue_num=qctr[0] % N_QUEUES,
                        )
                        qctr[0] += 1
                        subs.append(xg)
                    gathers[r] = (subs, first)

                psum = {b: psa.tile([128, 128], f32, tag="aggps",
                                    name=f"aggps{b}")
                        for b in blocks}
                done = {b: 0 for b in blocks}
                for r in range(N_RANGES):
                    if r not in gathers:
                        continue
                    subs, first = gathers[r]
                    for b in blocks:
                        for k in range(int(cpbr[b, r])):
                            j = int(chunk_off[b, r]) + k
                            off = j - first
                            xg = subs[off // GMAX]
                            col = off % GMAX
                            sv = svpool.tile([128, 128], bf16, tag="sv")
                            nc.vector.scalar_tensor_tensor(
                                out=sv[:],
                                in0=iota_sb[:],
                                scalar=dst_sb[:, j : j + 1],
                                in1=val_sb[:, j : j + 1]
                                    .to_broadcast([128, 128]),
                                op0=mybir.AluOpType.is_equal,
                                op1=mybir.AluOpType.mult,
                            )
                            nc.tensor.matmul(
                                out=psum[b][:],
                                lhsT=xg[:, col, :],
                                rhs=sv[:],
                                start=(done[b] == 0),
                                stop=(done[b] == int(nch_block[b]) - 1),
                            )
                            done[b] += 1
                            if done[b] == int(nch_block[b]):
                                agg_sb = aggpool.tile([128, 128], bf16,
                                                      tag="aggsb")
                                nc.scalar.activation(
                                    out=agg_sb[:], in_=psum[b][:],
                                    func=mybir.ActivationFunctionType.Copy,
                                )
                                out_ps = psb.tile([128, 128], f32, tag="outps")
                                nc.tensor.matmul(
                                    out=out_ps[:], lhsT=agg_sb[:], rhs=w_bf[:],
                                    start=True, stop=True,
                                )
                                out_sb = opool.tile([128, 128], f32, tag="osb")
                                nc.scalar.activation(
                                    out=out_sb[:], in_=out_ps[:],
                                    func=mybir.ActivationFunctionType.Relu,
                                )
                                nc.sync.dma_start(
                                    out=out_t.ap()[b * 128 : (b + 1) * 128, :],
                                    in_=out_sb[:],
                                )

    nc.compile()
    return nc


def _prep_inputs(input_features, weight, edge_vals, edge_src, edge_dst):
    cpbr, tot, idx16, dst_T, val_T = _schedule(
        np.asarray(edge_src), np.asarray(edge_vals), np.asarray(edge_dst)
    )
    x_bf = np.asarray(input_features).astype(ml_dtypes.bfloat16)
    w = np.ascontiguousarray(np.asarray(weight, dtype=np.float32))
    iota = np.tile(np.arange(128, dtype=np.float32), (128, 1)).astype(
        ml_dtypes.bfloat16)
    in_maps = []
    for c in range(N_CORES):
        in_maps.append({
            "xbf": x_bf,
            "w": w,
            "iota": iota,
            "idx": np.ascontiguousarray(idx16[c]),
            "dstrel": np.ascontiguousarray(dst_T[c]),
            "val": np.ascontiguousarray(val_T[c]),
        })
    return cpbr, tot, in_maps


def _program_key(prep):
    """Hashable _build_program key from a _prep_inputs result."""
    return tuple(int(x) for x in np.asarray(prep[0]).reshape(-1))


def kernel(input_features, weight, edge_vals, edge_src, edge_dst):
    prep = _prep_inputs(
        input_features, weight, edge_vals, edge_src, edge_dst
    )
    cpbr, tot, in_maps = prep
    nc = _build_program(_program_key(prep))
    res = run_bass_kernel_spmd(nc, in_maps, list(range(N_CORES)))
    out = np.concatenate(
        [res.results[c]["out"][:ROWS_PER_CORE] for c in range(N_CORES)], axis=0
    )
    return out.astype(np.float32)

